# revision 68
# baseline (speedup 1.0000x reference)
"""MLA attention (DeepSeek-style) on 8 TRN2 NeuronCores — unabsorbed 2D
(batch-pair x head) sharding.

Sharding: cores (2b, 2b+1) own batch b. Core 2b runs heads 0-7, core 2b+1
heads 8-15, each over the full 1024 tokens of its batch. Preprocessing
(c_q, c_kv, roped k_r latents) is token-split within the pair (512 tokens
each) and exchanged with ONE pair-group AllGather (~1.2 MB vs ~13 MB for
the old 8-way gather, which dominated runtime).

Math: instead of the absorbed per-head k_eff = U_q @ U_k (making attention
contract over the 512-dim latent), we factor back to per-head q_h = c_q@U_q,
k_h = U_k@c_kv, v_h = c_kv@v_eff_h (HS=128-dim attention contraction) —
~2x fewer PE MACs, and AV output lands directly in the head's output block
(v_eff_h = (W_uv.T @ W_o.T) head-block keeps the output per-head local, so
no all-reduce).

Precision: bf16 everywhere on the PE (2 cols/cycle streaming, FWL weight
loads); PSUM accumulation fp32. Attention is computed in the transposed
orientation logitsT[s, t] (no transposes anywhere; softmax needs no max
subtraction since logits are O(1)); column sums via an appended ones-row
matmul; host divides by the softmax denominators.
"""

import math

import numpy as np

B, T, C = 4, 1024, 2048
NH, HS = 16, 128
NLQ = NLKV = 512
DHR = 64
NCORES = 8
HPC = 8                     # heads per core (half the heads, own batch)
TOK = 512                   # tokens per core in preprocessing (half a batch)
SCALE = 1.0 / math.sqrt(HS + DHR)
CC = C // 128               # 16 contraction chunks over C

# pair-AllGather packed layout: [128, AGW] bf16 per core
COL_CQ = 0                  # c_qT_own   [128, 4qc x 512t]
COL_CKV = 4 * TOK           # c_kvT_own  [128, 4kc x 512t]
COL_KR = 8 * TOK            # k_rT_own   [64, 512] (duplicated rows 64:128)
AGW = 8 * TOK + TOK         # 4608

_cache = {}


def _build(loop_k=None, loop_pre=None, sim_single=False, phases="all",
           variant=""):
    """Build the SPMD kernel. loop_k / loop_pre: wrap the attention / the
    phase-0 body in a For_i hardware loop (timing amplification only).
    sim_single: single-core no-collective variant (gather output fed as an
    input) for CoreSim/TimelineSim analysis. phases="pre" drops phase 2
    (cost-model phase attribution only)."""
    import contextlib

    import concourse.mybir as mybir
    import concourse.tile as tile
    from concourse import bacc

    f32 = mybir.dt.float32
    bf16 = mybir.dt.bfloat16
    Exp = mybir.ActivationFunctionType.Exp
    Copy = mybir.ActivationFunctionType.Copy
    mult = mybir.AluOpType.mult
    add = mybir.AluOpType.add

    nc = bacc.Bacc(trn_type="TRN2", num_devices=1 if sim_single else NCORES)
    P = nc.declare_dram_parameter

    xT = P("xT", [128, CC * TOK], bf16, isOutput=False)
    wdqT = P("wdqT", [128, CC * NLQ], bf16, isOutput=False)
    wdkvT = P("wdkvT", [128, CC * NLKV], bf16, isOutput=False)
    wkr2T = P("wkr2T", [128, CC * 2 * DHR], bf16, isOutput=False)
    wuqT = P("wuqT", [128, HPC * 512], bf16, isOutput=False)
    wukT = P("wukT", [128, HPC * 512], bf16, isOutput=False)
    wqr2T = P("wqr2T", [128, HPC * 1024 // 2], bf16, isOutput=False)
    veffp = P("veffp", [128, 4 * HPC * HS], bf16, isOutput=False)
    cos2d = P("cos2d", [128, T], f32, isOutput=False)
    sin2d = P("sin2d", [128, T], f32, isOutput=False)
    cos2o = P("cos2o", [DHR, TOK], f32, isOutput=False)
    sin2o = P("sin2o", [DHR, TOK], f32, isOutput=False)
    maskp = P("maskp", [128, 128], bf16, isOutput=False)
    out = P("out", [HPC * HS, T], f32, isOutput=True)
    out2 = P("out2", [4 * HPC, T], f32, isOutput=True)
    ag_p = None
    if sim_single:
        ag_p = P("ag_p", [2 * 128, AGW], bf16, isOutput=False)

    with tile.TileContext(nc) as tc:
        with (
            tc.tile_pool(name="pres", bufs=1) as pres,
            tc.tile_pool(name="dram", bufs=1, space="DRAM") as dram,
            # 8 PSUM banks total: work 2x[128,1024]=4, proj 2x[128,512]=2,
            # av 1, sums 1
            tc.tile_pool(name="ps_work", bufs=2, space="PSUM") as ps_work,
            tc.tile_pool(name="ps_proj", bufs=2, space="PSUM") as ps_proj,
            tc.tile_pool(name="ps_av",
                         bufs=2 if "nosums" in variant else 1,
                         space="PSUM") as ps_av,
            tc.tile_pool(name="ps_sums", bufs=1, space="PSUM") as ps_sums,
        ):
            # ---------- resident tensors ----------
            wuqT_sb = pres.tile([128, HPC * 512], bf16, tag="wuqT")
            wukT_sb = pres.tile([128, HPC * 512], bf16, tag="wukT")
            wqr2T_sb = pres.tile([128, HPC * 512], bf16, tag="wqr2T")
            cos2d_sb = pres.tile([128, T], f32, tag="cos2d")
            sin2d_sb = pres.tile([128, T], f32, tag="sin2d")
            cos2o_sb = pres.tile([DHR, TOK], f32, tag="cos2o")
            sin2o_sb = pres.tile([DHR, TOK], f32, tag="sin2o")
            v_eff_sb = pres.tile([128, 4 * HPC * HS], bf16, tag="v_eff")
            cqT_f = pres.tile([128, 4 * T], bf16, tag="cqT_f")
            ckvT_f = pres.tile([128, 4 * T], bf16, tag="ckvT_f")
            krT_f = pres.tile([DHR, T], bf16, tag="krT_f")
            ones_sb = pres.tile([128, 1], f32, tag="ones")
            ones_r = pres.tile([128, 1], bf16, tag="ones_r")
            mask_r = pres.tile([128, 128], bf16, tag="mask_r")
            zro_sb = pres.tile([128, 512], f32, tag="zro")
            nc.gpsimd.memset(zro_sb[:], 0.0)

            nc.sync.dma_start(wuqT_sb[:], wuqT[:, :])
            nc.scalar.dma_start(wukT_sb[:], wukT[:, :])
            nc.sync.dma_start(wqr2T_sb[:], wqr2T[:, :])
            nc.scalar.dma_start(v_eff_sb[:], veffp[:, :])
            nc.sync.dma_start(cos2d_sb[:], cos2d[:])
            nc.scalar.dma_start(sin2d_sb[:], sin2d[:])
            nc.sync.dma_start(cos2o_sb[:], cos2o[:])
            nc.scalar.dma_start(sin2o_sb[:], sin2o[:])
            nc.gpsimd.memset(ones_sb[:], 1.0)
            nc.vector.tensor_copy(ones_r[:], ones_sb[:])
            nc.sync.dma_start(mask_r[:], maskp[:])

            # DRAM bounce buffers for the pair AllGather (Local output:
            # shared-output collectives need >4-core groups)
            agin = dram.tile([128, AGW], bf16)
            agout = ag_p if sim_single else dram.tile([2 * 128, AGW], bf16)

            # ---------- phase 0a: local latents + pair exchange ----------
            def phase0():
              with (
                tc.tile_pool(name="p0", bufs=1) as p0,
                tc.For_i(0, loop_pre, 1, hint_engines=(mybir.EngineType.PE,))
                if loop_pre else contextlib.nullcontext(),
              ):
                xT_sb = p0.tile([128, CC * TOK], bf16, tag="xT")
                wdqT_sb = p0.tile([128, CC * NLQ], bf16, tag="wdqT")
                wdkvT_sb = p0.tile([128, CC * NLKV], bf16, tag="wdkvT")
                wkr2T_sb = p0.tile([128, CC * 2 * DHR], bf16, tag="wkr2T")
                cq_loc = p0.tile([128, 4 * TOK], bf16, tag="cq_loc")
                ckv_loc = p0.tile([128, 4 * TOK], bf16, tag="ckv_loc")
                kr_loc = p0.tile([DHR, TOK], bf16, tag="kr_loc")
                rtmp = p0.tile([DHR, 2 * TOK], f32, tag="rtmp")

                # input loads split across both HWDGE rings (SP + ACT)
                for qr_ in range(4):
                    csl = slice(qr_ * 4 * TOK, (qr_ + 1) * 4 * TOK)
                    nc.sync.dma_start(xT_sb[:, csl], xT[:, csl])
                    wsl = slice(qr_ * 4 * NLQ, (qr_ + 1) * 4 * NLQ)
                    nc.scalar.dma_start(wdqT_sb[:, wsl], wdqT[:, wsl])
                    nc.sync.dma_start(wdkvT_sb[:, wsl], wdkvT[:, wsl])
                    ksl = slice(qr_ * 8 * DHR, (qr_ + 1) * 8 * DHR)
                    nc.scalar.dma_start(wkr2T_sb[:, ksl], wkr2T[:, ksl])

                # all 4 PSUM slots (2 pools x 2 bufs) held across cc-groups
                # so compute on DMA chunk g overlaps the chunk g+1 loads
                pqs = [(ps_proj if qt % 2 else ps_work).tile(
                    [128, TOK], f32, tag="proj" if qt % 2 else "work",
                    name=f"p0cq_{qt}")
                    for qt in range(4)]
                for g in range(4):
                    for qt in range(4):
                        for cc in range(4 * g, 4 * g + 4):
                            nc.tensor.matmul(
                                pqs[qt][:],
                                wdqT_sb[:, cc * NLQ + qt * 128: cc * NLQ + (qt + 1) * 128],
                                xT_sb[:, cc * TOK:(cc + 1) * TOK],
                                start=(cc == 0), stop=(cc == CC - 1))
                for qt in range(4):
                    nc.vector.tensor_copy(cq_loc[:, qt * TOK:(qt + 1) * TOK],
                                          pqs[qt][:])
                pks = [(ps_proj if kt % 2 else ps_work).tile(
                    [128, TOK], f32, tag="proj" if kt % 2 else "work",
                    name=f"p0ckv_{kt}")
                    for kt in range(4)]
                for g in range(4):
                    for kt in range(4):
                        for cc in range(4 * g, 4 * g + 4):
                            nc.tensor.matmul(
                                pks[kt][:],
                                wdkvT_sb[:, cc * NLKV + kt * 128: cc * NLKV + (kt + 1) * 128],
                                xT_sb[:, cc * TOK:(cc + 1) * TOK],
                                start=(cc == 0), stop=(cc == CC - 1))
                for kt in range(4):
                    nc.vector.tensor_copy(ckv_loc[:, kt * TOK:(kt + 1) * TOK],
                                          pks[kt][:])
                # roped k_r: rows 0..63 raw, 64..127 pair-swapped copy
                pr = ps_proj.tile([128, TOK], f32, tag="proj")
                for cc in range(CC):
                    nc.tensor.matmul(
                        pr[:],
                        wkr2T_sb[:, cc * 2 * DHR:(cc + 1) * 2 * DHR],
                        xT_sb[:, cc * TOK:(cc + 1) * TOK],
                        start=(cc == 0), stop=(cc == CC - 1))
                nc.vector.tensor_tensor(rtmp[:, :TOK], pr[:DHR, :], cos2o_sb[:], mult)
                nc.vector.tensor_tensor(rtmp[:, TOK:], pr[DHR:, :], sin2o_sb[:], mult)
                nc.vector.tensor_tensor(kr_loc[:], rtmp[:, :TOK], rtmp[:, TOK:], add)

                nc.gpsimd.dma_start(agin[:, COL_CQ:COL_CQ + 4 * TOK], cq_loc[:])
                nc.gpsimd.dma_start(agin[:, COL_CKV:COL_CKV + 4 * TOK], ckv_loc[:])
                nc.gpsimd.dma_start(agin[:DHR, COL_KR:COL_KR + TOK], kr_loc[:])
                nc.gpsimd.dma_start(agin[DHR:, COL_KR:COL_KR + TOK], kr_loc[:])

            if phases != "attn":
                phase0()

            if not sim_single and phases != "attn":
                nc.gpsimd.collective_compute(
                    "AllGather", mybir.AluOpType.bypass,
                    replica_groups=[[2 * i, 2 * i + 1] for i in range(NCORES // 2)],
                    ins=[agin.opt()], outs=[agout.opt()])

            # ---------- phase 1: unpack gathered latents ----------
            # cqT_f/ckvT_f are r-major: col = ts*2048 + qc*512 + t_loc, so
            # each rank's block is ONE contiguous DMA from the gather output
            # split unpack across both HWDGE rings (SP + ACT) for parallelism
            ag = agout.ap() if sim_single else agout[:]
            for r in range(2):
                rows = slice(r * 128, (r + 1) * 128)
                eng = nc.sync if r == 0 else nc.scalar
                eng.dma_start(
                    cqT_f[:, r * 4 * TOK:(r + 1) * 4 * TOK],
                    ag[rows, COL_CQ:COL_CQ + 4 * TOK])
                eng.dma_start(
                    ckvT_f[:, r * 4 * TOK:(r + 1) * 4 * TOK],
                    ag[rows, COL_CKV:COL_CKV + 4 * TOK])
                eng.dma_start(
                    krT_f[:, r * TOK:(r + 1) * TOK],
                    ag[r * 128: r * 128 + DHR, COL_KR:COL_KR + TOK])

            # ---------- phase 2: per-head projections + attention ----------
            with (
                tc.tile_pool(name="pv2", bufs=1) as pv2,
                tc.tile_pool(name="ph", bufs=2) as ph,
                tc.tile_pool(name="pqr", bufs=2) as pqr,
                tc.tile_pool(name="pex", bufs=14) as pex,
                tc.tile_pool(name="py", bufs=2) as py,
                tc.For_i(0, loop_k, 1, hint_engines=(mybir.EngineType.PE,))
                if loop_k else contextlib.nullcontext(),
            ):
                hpc_eff = 0 if phases == "pre" else HPC
                for tok in variant.split(","):
                    if tok.startswith("h") and tok[1:].isdigit():
                        hpc_eff = int(tok[1:])
                D2 = HPC * HS
                # v_s[s-chunk, d] for all 8 heads: [128, 8sc x (8h x 128d)]
                if hpc_eff:
                    v_s = pv2.tile([128, 8 * HPC * HS], bf16, tag="v_s")
                for sc in range(8 if hpc_eff else 0):
                    scb = (sc // 4) * 2048 + (sc % 4) * 128
                    pv = ps_work.tile([128, T], f32, tag="work")
                    for nn in range(2):
                        for kc in range(4):
                            nc.tensor.matmul(
                                pv[:, nn * 512:(nn + 1) * 512],
                                ckvT_f[:, scb + kc * 512: scb + kc * 512 + 128],
                                v_eff_sb[:, kc * D2 + nn * 512: kc * D2 + (nn + 1) * 512],
                                start=(kc == 0), stop=(kc == 3))
                    nc.vector.tensor_copy(v_s[:, sc * D2:(sc + 1) * D2], pv[:])

                for hh in range(hpc_eff):
                    # q_hT[d, t], k_hT[d, s] for the full batch
                    qh = ph.tile([128, T], bf16, tag="qh")
                    kh = ph.tile([128, T], bf16, tag="kh")
                    for ts in range(2):
                        pq = ps_proj.tile([128, 512], f32, tag="proj")
                        for qc in range(4):
                            nc.tensor.matmul(
                                pq[:],
                                wuqT_sb[:, hh * 512 + qc * 128: hh * 512 + (qc + 1) * 128],
                                cqT_f[:, ts * 2048 + qc * 512: ts * 2048 + (qc + 1) * 512],
                                start=(qc == 0), stop=(qc == 3))
                        nc.vector.tensor_copy(qh[:, ts * 512:(ts + 1) * 512], pq[:])
                    for ts in range(2):
                        pk = ps_proj.tile([128, 512], f32, tag="proj")
                        for kc in range(4):
                            nc.tensor.matmul(
                                pk[:],
                                wukT_sb[:, hh * 512 + kc * 128: hh * 512 + (kc + 1) * 128],
                                ckvT_f[:, ts * 2048 + kc * 512: ts * 2048 + (kc + 1) * 512],
                                start=(kc == 0), stop=(kc == 3))
                        nc.vector.tensor_copy(kh[:, ts * 512:(ts + 1) * 512], pk[:])
                    # roped q_r for a PAIR of heads at once (tile rows:
                    # head-even 0:64, head-odd 64:128), split on evac
                    if hh % 2 == 0:
                        pp = hh // 2
                        qrA = pqr.tile([DHR, T], bf16, tag="qrA")
                        qrB = pqr.tile([DHR, T], bf16, tag="qrB")
                        qrtmp = pqr.tile([128, 2 * 512], bf16, tag="qrtmp")
                        for ts in range(2):
                            pr2 = ps_work.tile([128, T], f32, tag="work")
                            for qc in range(4):
                                nc.tensor.matmul(
                                    pr2[:, :512],
                                    wqr2T_sb[:, pp * 1024 + qc * 128: pp * 1024 + (qc + 1) * 128],
                                    cqT_f[:, ts * 2048 + qc * 512: ts * 2048 + (qc + 1) * 512],
                                    start=(qc == 0), stop=(qc == 3))
                            for qc in range(4):
                                nc.tensor.matmul(
                                    pr2[:, 512:],
                                    wqr2T_sb[:, pp * 1024 + 512 + qc * 128: pp * 1024 + 512 + (qc + 1) * 128],
                                    cqT_f[:, ts * 2048 + qc * 512: ts * 2048 + (qc + 1) * 512],
                                    start=(qc == 0), stop=(qc == 3))
                            tsl = slice(ts * 512, (ts + 1) * 512)
                            nc.vector.tensor_tensor(
                                qrtmp[:, :512], pr2[:, :512], cos2d_sb[:, tsl], mult)
                            nc.vector.tensor_tensor(
                                qrtmp[:, 512:], pr2[:, 512:], sin2d_sb[:, tsl], mult)
                            nc.vector.tensor_tensor(
                                qrA[:, tsl], qrtmp[:DHR, :512], qrtmp[:DHR, 512:], add)
                            nc.vector.tensor_tensor(
                                qrB[:, tsl], qrtmp[DHR:, :512], qrtmp[DHR:, 512:], add)
                    qr = qrA if hh % 2 == 0 else qrB

                    y_sb = py.tile([128, T], f32, tag="y")
                    sums_sb = py.tile([97, T], f32, tag="sums")
                    # pass 1: logits + exp, 8 s-chunks over the full t-range
                    # (one [128, njt] exp per chunk; lg spans 2 PSUM banks)
                    exs = []
                    for j in range(8):
                        t_off = 128 * j
                        lg = ps_work.tile([128, T], f32, tag="work")
                        for th in range(2):
                            c0 = max(t_off, th * 512)
                            if c0 >= (th + 1) * 512:
                                continue
                            csl = slice(c0, (th + 1) * 512)
                            nc.tensor.matmul(
                                lg[:, csl], kh[:, j * 128:(j + 1) * 128],
                                qh[:, csl], start=True, stop=False)
                            nc.tensor.matmul(
                                lg[:, csl], krT_f[:, j * 128:(j + 1) * 128],
                                qr[:, csl], start=False, stop=True)
                        ex = pex.tile([128, T], bf16, tag="ex",
                                      name=f"ex_{hh}_{j}")
                        nc.scalar.activation(ex[:, t_off:], lg[:, t_off:],
                                             Exp, scale=SCALE)
                        nc.gpsimd.tensor_tensor(
                            ex[:, t_off:t_off + 128], ex[:, t_off:t_off + 128],
                            mask_r[:], mult)
                        exs.append(ex)
                    # pass 2: AV + sums accumulation per t-half bank
                    nosums = "nosums" in variant
                    for th in range(2):
                        av_ps = ps_av.tile([128, 512], f32, tag="av")
                        if not nosums:
                            # sums via 4-way col-tiled concurrent M=1 MMs
                            # (partials on partitions 0/32/64/96; host adds
                            # the rows). The bank is DVE-zeroed first, so
                            # overwrite-where-unwritten vs accumulate are
                            # numerically identical -> start=False is sound.
                            sums_ps = ps_sums.tile([128, 512], f32, tag="sums")
                            nc.vector.tensor_copy(sums_ps[:], zro_sb[:])
                        njs = [j for j in range(8) if 128 * j < (th + 1) * 512]
                        for i, j in enumerate(njs):
                            c0 = max(128 * j, th * 512)
                            esl = slice(c0, (th + 1) * 512)
                            osl = slice(c0 - th * 512, 512)
                            first, last = (i == 0), (i == len(njs) - 1)
                            nc.tensor.matmul(
                                av_ps[:, osl],
                                v_s[:, j * D2 + hh * HS: j * D2 + (hh + 1) * HS],
                                exs[j][:, esl], start=first, stop=last)
                            if not nosums:
                                cp = 32 * (j % 4)
                                nc.tensor.matmul(
                                    sums_ps[cp:cp + 1, osl], ones_r[:],
                                    exs[j][:, esl],
                                    start=False, stop=(j + 4 not in njs),
                                    tile_position=(0, cp),
                                    skip_group_check=True)
                        # y evac on ACT: DVE is nearer the critical path
                        # (it owns the q/k/v/qr evacs)
                        tsl = slice(th * 512, (th + 1) * 512)
                        nc.scalar.activation(y_sb[:, tsl], av_ps[:], Copy)
                        if not nosums:
                            nc.vector.tensor_copy(sums_sb[:, tsl],
                                                  sums_ps[:97, :])
                    nc.sync.dma_start(out[hh * HS:(hh + 1) * HS, :], y_sb[:])
                    if "nosums" not in variant:
                        for c in range(4):
                            nc.scalar.dma_start(out2[4 * hh + c: 4 * hh + c + 1, :],
                                                sums_sb[32 * c: 32 * c + 1, :])
    nc.compile()
    return nc


def _pairswap(w):
    idx = np.arange(w.shape[0]).reshape(-1, 2)[:, ::-1].reshape(-1)
    return w[idx]


def _slab(m, dtype):
    """[n*128, W] row-major -> SBUF slab layout [128, n*W]."""
    n = m.shape[0] // 128
    return np.ascontiguousarray(
        m.reshape(n, 128, m.shape[1]).transpose(1, 0, 2).reshape(128, -1),
        dtype=dtype)


def _make_in_maps(x, W_dq, W_uq, W_dkv, W_uk, W_uv, W_o, W_qr, W_kr,
                  freqs_cos, freqs_sin):
    import ml_dtypes
    f4 = np.float32
    bf = ml_dtypes.bfloat16
    wdqT = _slab(W_dq.T, bf)
    wdkvT = _slab(W_dkv.T, bf)
    wkr2T = _slab(np.concatenate([W_kr.T, _pairswap(W_kr).T], axis=1), bf)
    veff_full = W_uv.T.astype(f4) @ W_o.T.astype(f4)   # (NLKV, C) on host
    uq = W_uq.reshape(NLQ, NH, HS)
    uk = W_uk.reshape(NH, HS, NLKV)
    cos2 = np.repeat(freqs_cos.T, 2, axis=0).astype(f4)          # [DHR, T]
    sin_half = freqs_sin.T.astype(f4)                            # [DHR/2, T]
    sin2 = np.empty((DHR, T), dtype=f4)
    sin2[0::2] = -sin_half
    sin2[1::2] = sin_half
    cos2d = np.concatenate([cos2, cos2], axis=0)                 # [128, T]
    sin2d = np.concatenate([sin2, sin2], axis=0)

    in_maps = []
    for i in range(NCORES):
        b_own, half = divmod(i, 2)
        t0 = half * TOK
        heads = [HPC * half + hh for hh in range(HPC)]
        # per-head lhsT slabs: block hh at cols [hh*512 + qc*128]
        wuqT = np.concatenate(
            [_slab(uq[:, h, :], bf) for h in heads], axis=1)     # A_h (NLQ, HS)
        wukT = np.concatenate(
            [_slab(uk[h].T, bf) for h in heads], axis=1)         # B_h.T (NLKV, HS)
        qr_tiles = []
        for p in range(HPC // 2):
            hA, hB = heads[2 * p], heads[2 * p + 1]
            t1 = np.concatenate([W_qr[hA * DHR:(hA + 1) * DHR].T,
                                 W_qr[hB * DHR:(hB + 1) * DHR].T], axis=1)
            t2 = np.concatenate([_pairswap(W_qr[hA * DHR:(hA + 1) * DHR]).T,
                                 _pairswap(W_qr[hB * DHR:(hB + 1) * DHR]).T],
                                axis=1)
            qr_tiles += [t1, t2]
        wqr2T = np.concatenate([_slab(t, bf) for t in qr_tiles], axis=1)
        veffp = _slab(veff_full[:, heads[0] * HS:(heads[-1] + 1) * HS], bf)
        in_maps.append({
            "xT": _slab(x[b_own, t0:t0 + TOK, :].T, bf),
            "wdqT": wdqT, "wdkvT": wdkvT, "wkr2T": wkr2T,
            "wuqT": wuqT, "wukT": wukT, "wqr2T": wqr2T,
            "veffp": veffp,
            "cos2d": cos2d, "sin2d": sin2d,
            "cos2o": np.ascontiguousarray(cos2[:, t0:t0 + TOK]),
            "sin2o": np.ascontiguousarray(sin2[:, t0:t0 + TOK]),
            "maskp": np.triu(np.ones((128, 128))).astype(bf),
        })
    return in_maps


def _assemble(results):
    y = np.empty((B, T, C), dtype=np.float32)
    for i in range(NCORES):
        b_own, half = divmod(i, 2)
        o = results[i]["out"]    # [HPC*HS, T] (unnormalized)
        s2 = results[i]["out2"]  # [4*HPC, T] partial softmax denominators
        for hh in range(HPC):
            h = HPC * half + hh
            den = s2[4 * hh: 4 * hh + 4].sum(axis=0)
            blk = o[hh * HS:(hh + 1) * HS, :] / den
            y[b_own, :, h * HS:(h + 1) * HS] = blk.T
    return y


def kernel(**inputs):
    from concourse import bass_utils
    if "nc" not in _cache:
        _cache["nc"] = _build()
    nc = _cache["nc"]
    in_maps = _make_in_maps(**{k: np.asarray(v) for k, v in inputs.items()})
    res = bass_utils.run_bass_kernel_spmd(nc, in_maps, core_ids=list(range(NCORES)))
    return _assemble(res.results)
